# revision 2
# baseline (speedup 1.0000x reference)
"""Trainium2 Bass kernel for nn_LinearCrossAttention (B=4, N=4096, CH=(64,128,256,512)).

Math (per sample b, branch i with C channels):
    q      = x_total @ q_w.T + q_b                         [N]
    scores = softmax(q)  over tokens                       [N]
    kv     = x @ kv_w.T + kv_b ; k, v = split(kv)          [N, C] each
    ctx    = sum_n scores[n] * k[n, :]                     [C]
    o      = (relu(v) * ctx) @ out_w.T + out_b             [N, C]

Because scores sums to 1, ctx = kv_w[:C] @ (x.T @ scores) + kv_b[:C] — the
k-projection of every token is never needed, only the pooled xbar = x.T @
scores. And ctx folds into the output projection weights:
    o = relu(v) @ W' + out_b,   W'[c, d] = out_w[d, c] * ctx[c]

Device work = the two big matmuls (v-projection + out-projection, ~22.8
GFLOP total), computed in transposed layout (channels on partitions) so no
on-device transposes are needed. The tiny stats (q, softmax, xbar, ctx,
weight scaling — ~0.3% of FLOPs) run on host while sharding.

Sharding: 8 cores = 4 samples x 2 token-halves. Per core (sample b, half h):
    vT = relu(wvT.T @ xT + bv)      [C, 2048]   (wvT = kv_w[C:2C].T)
    oT = wp.T @ vT                  [C, 2048]   (wp[c,d] = out_w[d,c]*ctx[b,c])
Host gathers oT halves, transposes, adds out_b.

Matmuls run in bf16 (fp32 PSUM accumulation). The out-projection weights are
~0.02-scale and ctx multiplies them further down, so bf16 input rounding
contributes ~1e-5 absolute error against an output scale of ~0.1.
"""

import numpy as np
import ml_dtypes

import concourse.bass as bass
import concourse.mybir as mybir
import concourse.tile as tile
from concourse import bacc
from concourse.bass_utils import run_bass_kernel_spmd

B, N = 4, 4096
CH = (64, 128, 256, 512)
NT = N // 2  # tokens per core
P = 128
FD = 512  # matmul moving free dim == one fp32 PSUM bank
N_CORES = 8

BF16 = mybir.dt.bfloat16
F32 = mybir.dt.float32
np_bf16 = ml_dtypes.bfloat16

_CACHE = {}


def _build_nc():
    """Build the (single, SPMD) Bass program: per-core v-proj + out-proj."""
    nc = bacc.Bacc(None, target_bir_lowering=False, debug=False)

    dram = {}
    for i, C in enumerate(CH, start=1):
        dram[f"xT{i}"] = nc.dram_tensor(f"xT{i}", [C, NT], BF16, kind="ExternalInput")
        dram[f"wvT{i}"] = nc.dram_tensor(f"wvT{i}", [C, C], BF16, kind="ExternalInput")
        dram[f"wp{i}"] = nc.dram_tensor(f"wp{i}", [C, C], BF16, kind="ExternalInput")
        dram[f"bv{i}"] = nc.dram_tensor(f"bv{i}", [max(C // P, 1), min(C, P)], F32,
                                        kind="ExternalInput")
        dram[f"oT{i}"] = nc.dram_tensor(f"oT{i}", [C, NT], BF16, kind="ExternalOutput")

    with tile.TileContext(nc) as tc:
        with (
            tc.tile_pool(name="xpool", bufs=1) as xpool,
            tc.tile_pool(name="wpool", bufs=1) as wpool,
            tc.tile_pool(name="rvpool", bufs=1) as rvpool,
            tc.tile_pool(name="opool", bufs=1) as opool,
            tc.tile_pool(name="psum", bufs=8, space="PSUM") as psum,
        ):
            # Load + stage-1 (v-proj) emitters per branch, big branches first
            order = [4, 3, 2, 1]
            sb = {}
            for i in order:
                C = CH[i - 1]
                kt = max(C // P, 1)   # k/m tiles of 128 (1 tile of 64 for C=64)
                ps = min(C, P)        # partitions per tile
                xT = xpool.tile([ps, kt, NT], BF16, name=f"xT{i}_sb", tag=f"xT{i}")
                wv = wpool.tile([ps, kt, C], BF16, name=f"wvT{i}_sb", tag=f"wv{i}")
                wp = wpool.tile([ps, kt, C], BF16, name=f"wp{i}_sb", tag=f"wp{i}")
                bv = wpool.tile([ps, kt], F32, name=f"bv{i}_sb", tag=f"bv{i}")
                rv = rvpool.tile([ps, kt, NT], BF16, name=f"rv{i}_sb", tag=f"rv{i}")
                osb = opool.tile([ps, kt, NT], BF16, name=f"o{i}_sb", tag=f"o{i}")
                nc.sync.dma_start(wv[:], dram[f"wvT{i}"].rearrange(
                    "(k p) c -> p k c", p=ps))
                nc.sync.dma_start(bv[:], dram[f"bv{i}"].rearrange("k p -> p k"))
                nc.sync.dma_start(xT[:], dram[f"xT{i}"].rearrange(
                    "(k p) n -> p k n", p=ps))
                nc.sync.dma_start(wp[:], dram[f"wp{i}"].rearrange(
                    "(k p) c -> p k c", p=ps))
                sb[i] = (C, kt, ps, xT, wv, wp, bv, rv, osb)

            def stage1(i):
                C, kt, ps, xT, wv, wp, bv, rv, osb = sb[i]
                nj = NT // FD
                for m in range(kt):
                    pm = min(C - m * P, P)
                    pst = [psum.tile([P, FD], F32, name=f"ps1_{i}_{m}_{j}", tag="ps")
                           for j in range(nj)]
                    for k in range(kt):
                        lhsT = wv[:, k, m * P:m * P + pm]
                        for j in range(nj):
                            nc.tensor.matmul(
                                pst[j][:pm, :], lhsT, xT[:, k, j * FD:(j + 1) * FD],
                                start=(k == 0), stop=(k == kt - 1))
                    for j in range(nj):
                        nc.scalar.activation(
                            rv[:pm, m, j * FD:(j + 1) * FD], pst[j][:pm, :],
                            mybir.ActivationFunctionType.Relu,
                            bias=bv[:pm, m:m + 1])

            def stage2(i):
                C, kt, ps, xT, wv, wp, bv, rv, osb = sb[i]
                nj = NT // FD
                for m in range(kt):
                    pm = min(C - m * P, P)
                    pst = [psum.tile([P, FD], F32, name=f"ps2_{i}_{m}_{j}", tag="ps")
                           for j in range(nj)]
                    for k in range(kt):
                        lhsT = wp[:, k, m * P:m * P + pm]
                        for j in range(nj):
                            nc.tensor.matmul(
                                pst[j][:pm, :], lhsT, rv[:, k, j * FD:(j + 1) * FD],
                                start=(k == 0), stop=(k == kt - 1))
                    for j in range(nj):
                        nc.vector.tensor_copy(
                            osb[:pm, m, j * FD:(j + 1) * FD], pst[j][:pm, :])
                nc.sync.dma_start(
                    dram[f"oT{i}"].rearrange("(k p) n -> p k n", p=ps), osb[:])

            # Interleave stages so the PE never waits on an eviction chain:
            # while branch i's stage-2 deps (rv) finish, the next branch's
            # stage-1 matmuls keep the array busy.
            stage1(4)
            stage1(3)
            stage2(4)
            stage1(2)
            stage2(3)
            stage1(1)
            stage2(2)
            stage2(1)

    nc.compile()
    return nc


def _host_stats(inputs):
    """q -> softmax -> xbar -> ctx, all fp32 on host (~0.3% of FLOPs)."""
    xt = np.asarray(inputs["x_total"], dtype=np.float32)
    q = np.einsum("bnc,c->bn", xt, np.asarray(inputs["q_w"])[0],
                  optimize=True) + np.asarray(inputs["q_b"])[0]
    q -= q.max(axis=1, keepdims=True)
    e = np.exp(q)
    scores = e / e.sum(axis=1, keepdims=True)  # [B, N]
    ctxs = {}
    for i, C in enumerate(CH, start=1):
        x = np.asarray(inputs[f"x{i}"], dtype=np.float32)
        kvw = np.asarray(inputs[f"kv{i}_w"])
        kvb = np.asarray(inputs[f"kv{i}_b"])
        xbar = np.einsum("bnc,bn->bc", x, scores, optimize=True)  # [B, C]
        ctxs[i] = xbar @ kvw[:C].T + kvb[:C]  # [B, C]
    return ctxs


def kernel(**inputs):
    if "nc" not in _CACHE:
        _CACHE["nc"] = _build_nc()
    nc = _CACHE["nc"]

    ctxs = _host_stats(inputs)

    # Shared (per-branch, all-core) tensors
    shared = {}
    for i, C in enumerate(CH, start=1):
        kvw = np.asarray(inputs[f"kv{i}_w"])
        kvb = np.asarray(inputs[f"kv{i}_b"]).astype(np.float32)
        shared[f"wvT{i}"] = np.ascontiguousarray(kvw[C:].T).astype(np_bf16)
        shared[f"bv{i}"] = np.ascontiguousarray(
            kvb[C:].reshape(max(C // P, 1), min(C, P)))

    in_maps = []
    for core in range(N_CORES):
        b, h = divmod(core, 2)
        m = dict(shared)
        for i, C in enumerate(CH, start=1):
            x = np.asarray(inputs[f"x{i}"])
            ow = np.asarray(inputs[f"out{i}_w"])
            m[f"xT{i}"] = np.ascontiguousarray(
                x[b, h * NT:(h + 1) * NT, :].T).astype(np_bf16)
            m[f"wp{i}"] = (ow.T * ctxs[i][b][:, None]).astype(np_bf16)
        in_maps.append(m)

    _CACHE["in_maps"] = in_maps
    res = run_bass_kernel_spmd(nc, in_maps, list(range(N_CORES)))
    _CACHE["last_results"] = res

    outs = []
    for i, C in enumerate(CH, start=1):
        ob = np.asarray(inputs[f"out{i}_b"]).astype(np.float32)
        per_b = []
        for b in range(B):
            halves = [res.results[2 * b + h][f"oT{i}"].astype(np.float32)
                      for h in range(2)]
            per_b.append(np.concatenate(halves, axis=1).T + ob)  # [N, C]
        outs.append(np.stack(per_b).astype(np.float32))  # [B, N, C]

    return (outs[0], outs[1], outs[2], outs[3], [])


# revision 3
# speedup vs baseline: 1.1148x; 1.1148x over previous
"""Trainium2 Bass kernel for nn_LinearCrossAttention (B=4, N=4096, CH=(64,128,256,512)).

Math (per sample b, branch i with C channels):
    q      = x_total @ q_w.T + q_b                         [N]
    scores = softmax(q)  over tokens                       [N]
    kv     = x @ kv_w.T + kv_b ; k, v = split(kv)          [N, C] each
    ctx    = sum_n scores[n] * k[n, :]                     [C]
    o      = (relu(v) * ctx) @ out_w.T + out_b             [N, C]

Because scores sums to 1, ctx = kv_w[:C] @ (x.T @ scores) + kv_b[:C] — the
k-projection of every token is never needed, only the pooled xbar = x.T @
scores. And ctx folds into the output projection weights:
    o = relu(v) @ W' + out_b,   W'[c, d] = out_w[d, c] * ctx[c]

Device work = the two big matmuls (v-projection + out-projection, ~22.8
GFLOP total), computed in transposed layout (channels on partitions) so no
on-device transposes are needed. The tiny stats (q, softmax, xbar, ctx,
weight scaling — ~0.3% of FLOPs) run on host while sharding.

Sharding: 8 cores = 4 samples x 2 token-halves. Per core (sample b, half h):
    vT = relu(wvT.T @ xT + bv)      [C, 2048]   (wvT = kv_w[C:2C].T)
    oT = wp.T @ vT                  [C, 2048]   (wp[c,d] = out_w[d,c]*ctx[b,c])
Host gathers oT halves, transposes, adds out_b.

Matmuls run in bf16 (fp32 PSUM accumulation). The out-projection weights are
~0.02-scale and ctx multiplies them further down, so bf16 input rounding
contributes ~1e-5 absolute error against an output scale of ~0.1.
"""

import numpy as np
import ml_dtypes

import concourse.bass as bass
import concourse.mybir as mybir
import concourse.tile as tile
from concourse import bacc
from concourse.bass_utils import run_bass_kernel_spmd

B, N = 4, 4096
CH = (64, 128, 256, 512)
NT = N // 2  # tokens per core
P = 128
FD = 512  # matmul moving free dim == one fp32 PSUM bank
N_CORES = 8

BF16 = mybir.dt.bfloat16
F32 = mybir.dt.float32
np_bf16 = ml_dtypes.bfloat16

_CACHE = {}


def _build_nc():
    """Build the (single, SPMD) Bass program: per-core v-proj + out-proj."""
    nc = bacc.Bacc(None, target_bir_lowering=False, debug=False)

    dram = {}
    for i, C in enumerate(CH, start=1):
        dram[f"xT{i}"] = nc.dram_tensor(f"xT{i}", [C, NT], BF16, kind="ExternalInput")
        dram[f"wvT{i}"] = nc.dram_tensor(f"wvT{i}", [C, C], BF16, kind="ExternalInput")
        dram[f"wp{i}"] = nc.dram_tensor(f"wp{i}", [C, C], BF16, kind="ExternalInput")
        dram[f"bv{i}"] = nc.dram_tensor(f"bv{i}", [max(C // P, 1), min(C, P)], F32,
                                        kind="ExternalInput")
        dram[f"oT{i}"] = nc.dram_tensor(f"oT{i}", [C, NT], BF16, kind="ExternalOutput")

    with tile.TileContext(nc) as tc:
        with (
            tc.tile_pool(name="xpool", bufs=1) as xpool,
            tc.tile_pool(name="wpool", bufs=1) as wpool,
            tc.tile_pool(name="rvpool", bufs=1) as rvpool,
            tc.tile_pool(name="opool", bufs=1) as opool,
            tc.tile_pool(name="psum", bufs=8, space="PSUM") as psum,
        ):
            order = [4, 3, 2, 1]  # big branches first
            sb = {}
            for i in order:
                C = CH[i - 1]
                kt = max(C // P, 1)   # k/m tiles of 128 (1 tile of 64 for C=64)
                ps = min(C, P)        # partitions per tile
                xT = xpool.tile([ps, kt, NT], BF16, name=f"xT{i}_sb", tag=f"xT{i}")
                wv = wpool.tile([ps, kt, C], BF16, name=f"wvT{i}_sb", tag=f"wv{i}")
                wp = wpool.tile([ps, kt, C], BF16, name=f"wp{i}_sb", tag=f"wp{i}")
                bv = wpool.tile([ps, kt], F32, name=f"bv{i}_sb", tag=f"bv{i}")
                rv = rvpool.tile([ps, kt, NT], BF16, name=f"rv{i}_sb", tag=f"rv{i}")
                osb = opool.tile([ps, kt, NT], BF16, name=f"o{i}_sb", tag=f"o{i}")
                sb[i] = (C, kt, ps, xT, wv, wp, bv, rv, osb)

            # Input DMA order = consumption order: tiny biases, then per-k-tile
            # chunks of (weights, activations) so the first matmul's deps are
            # ~256KB, not the whole 5MB input set (HWDGE ring is FIFO).
            for i in order:
                C, kt, ps, xT, wv, wp, bv, rv, osb = sb[i]
                nc.sync.dma_start(bv[:], dram[f"bv{i}"].rearrange("k p -> p k"))
            for i in order:
                C, kt, ps, xT, wv, wp, bv, rv, osb = sb[i]
                for k in range(kt):
                    rs = slice(k * ps, (k + 1) * ps)
                    nc.sync.dma_start(wv[:, k, :], dram[f"wvT{i}"][rs, :])
                    nc.sync.dma_start(xT[:, k, :], dram[f"xT{i}"][rs, :])
            for i in order:
                C, kt, ps, xT, wv, wp, bv, rv, osb = sb[i]
                for k in range(kt):
                    rs = slice(k * ps, (k + 1) * ps)
                    nc.sync.dma_start(wp[:, k, :], dram[f"wp{i}"][rs, :])

            # j-outer, k-inner, m-innermost: the first PSUM group only needs
            # the k0 chunk to start, and each arriving k chunk immediately
            # feeds `kt` matmuls.
            def stage1(i):
                C, kt, ps, xT, wv, wp, bv, rv, osb = sb[i]
                nj = NT // FD
                for j in range(nj):
                    js = slice(j * FD, (j + 1) * FD)
                    pst = [psum.tile([P, FD], F32, name=f"ps1_{i}_{j}_{m}", tag="ps")
                           for m in range(kt)]
                    for k in range(kt):
                        for m in range(kt):
                            pm = min(C - m * P, P)
                            nc.tensor.matmul(
                                pst[m][:pm, :], wv[:, k, m * P:m * P + pm],
                                xT[:, k, js],
                                start=(k == 0), stop=(k == kt - 1))
                    for m in range(kt):
                        pm = min(C - m * P, P)
                        nc.scalar.activation(
                            rv[:pm, m, js], pst[m][:pm, :],
                            mybir.ActivationFunctionType.Relu,
                            bias=bv[:pm, m:m + 1])

            def stage2(i):
                C, kt, ps, xT, wv, wp, bv, rv, osb = sb[i]
                nj = NT // FD
                oT_d = dram[f"oT{i}"].rearrange("(k p) n -> p k n", p=ps)
                for j in range(nj):
                    js = slice(j * FD, (j + 1) * FD)
                    pst = [psum.tile([P, FD], F32, name=f"ps2_{i}_{j}_{m}", tag="ps")
                           for m in range(kt)]
                    for k in range(kt):
                        for m in range(kt):
                            pm = min(C - m * P, P)
                            nc.tensor.matmul(
                                pst[m][:pm, :], wp[:, k, m * P:m * P + pm],
                                rv[:, k, js],
                                start=(k == 0), stop=(k == kt - 1))
                    for m in range(kt):
                        pm = min(C - m * P, P)
                        nc.vector.tensor_copy(osb[:pm, m, js], pst[m][:pm, :])
                    nc.sync.dma_start(oT_d[:, :, js], osb[:, :, js])

            # Interleave stages so the PE never waits on an eviction chain:
            # while branch i's stage-2 deps (rv) finish, the next branch's
            # stage-1 matmuls keep the array busy.
            stage1(4)
            stage1(3)
            stage2(4)
            stage1(2)
            stage2(3)
            stage1(1)
            stage2(2)
            stage2(1)

    nc.compile()
    return nc


def _host_stats(inputs):
    """q -> softmax -> xbar -> ctx, all fp32 on host (~0.3% of FLOPs)."""
    xt = np.asarray(inputs["x_total"], dtype=np.float32)
    q = np.einsum("bnc,c->bn", xt, np.asarray(inputs["q_w"])[0],
                  optimize=True) + np.asarray(inputs["q_b"])[0]
    q -= q.max(axis=1, keepdims=True)
    e = np.exp(q)
    scores = e / e.sum(axis=1, keepdims=True)  # [B, N]
    ctxs = {}
    for i, C in enumerate(CH, start=1):
        x = np.asarray(inputs[f"x{i}"], dtype=np.float32)
        kvw = np.asarray(inputs[f"kv{i}_w"])
        kvb = np.asarray(inputs[f"kv{i}_b"])
        xbar = np.einsum("bnc,bn->bc", x, scores, optimize=True)  # [B, C]
        ctxs[i] = xbar @ kvw[:C].T + kvb[:C]  # [B, C]
    return ctxs


def kernel(**inputs):
    if "nc" not in _CACHE:
        _CACHE["nc"] = _build_nc()
    nc = _CACHE["nc"]

    ctxs = _host_stats(inputs)

    # Shared (per-branch, all-core) tensors
    shared = {}
    for i, C in enumerate(CH, start=1):
        kvw = np.asarray(inputs[f"kv{i}_w"])
        kvb = np.asarray(inputs[f"kv{i}_b"]).astype(np.float32)
        shared[f"wvT{i}"] = np.ascontiguousarray(kvw[C:].T).astype(np_bf16)
        shared[f"bv{i}"] = np.ascontiguousarray(
            kvb[C:].reshape(max(C // P, 1), min(C, P)))

    in_maps = []
    for core in range(N_CORES):
        b, h = divmod(core, 2)
        m = dict(shared)
        for i, C in enumerate(CH, start=1):
            x = np.asarray(inputs[f"x{i}"])
            ow = np.asarray(inputs[f"out{i}_w"])
            m[f"xT{i}"] = np.ascontiguousarray(
                x[b, h * NT:(h + 1) * NT, :].T).astype(np_bf16)
            m[f"wp{i}"] = (ow.T * ctxs[i][b][:, None]).astype(np_bf16)
        in_maps.append(m)

    _CACHE["in_maps"] = in_maps
    res = run_bass_kernel_spmd(nc, in_maps, list(range(N_CORES)))
    _CACHE["last_results"] = res

    outs = []
    for i, C in enumerate(CH, start=1):
        ob = np.asarray(inputs[f"out{i}_b"]).astype(np.float32)
        per_b = []
        for b in range(B):
            halves = [res.results[2 * b + h][f"oT{i}"].astype(np.float32)
                      for h in range(2)]
            per_b.append(np.concatenate(halves, axis=1).T + ob)  # [N, C]
        outs.append(np.stack(per_b).astype(np.float32))  # [B, N, C]

    return (outs[0], outs[1], outs[2], outs[3], [])


# revision 4
# speedup vs baseline: 1.1690x; 1.0486x over previous
"""Trainium2 Bass kernel for nn_LinearCrossAttention (B=4, N=4096, CH=(64,128,256,512)).

Math (per sample b, branch i with C channels):
    q      = x_total @ q_w.T + q_b                         [N]
    scores = softmax(q)  over tokens                       [N]
    kv     = x @ kv_w.T + kv_b ; k, v = split(kv)          [N, C] each
    ctx    = sum_n scores[n] * k[n, :]                     [C]
    o      = (relu(v) * ctx) @ out_w.T + out_b             [N, C]

Because scores sums to 1, ctx = kv_w[:C] @ (x.T @ scores) + kv_b[:C] — the
k-projection of every token is never needed, only the pooled xbar = x.T @
scores. And ctx folds into the output projection weights:
    o = relu(v) @ W' + out_b,   W'[c, d] = out_w[d, c] * ctx[c]

Device work = the two big matmuls (v-projection + out-projection, ~22.8
GFLOP total), computed in transposed layout (channels on partitions) so no
on-device transposes are needed. The tiny stats (q, softmax, xbar, ctx,
weight scaling — ~0.3% of FLOPs) run on host while sharding.

Sharding: 8 cores = 4 samples x 2 token-halves. Per core (sample b, half h):
    vT = relu(wvT.T @ xT + bv)      [C, 2048]   (wvT = kv_w[C:2C].T)
    oT = wp.T @ vT                  [C, 2048]   (wp[c,d] = out_w[d,c]*ctx[b,c])
Host gathers oT halves, transposes, adds out_b.

Matmuls run in bf16 (fp32 PSUM accumulation). The out-projection weights are
~0.02-scale and ctx multiplies them further down, so bf16 input rounding
contributes ~1e-5 absolute error against an output scale of ~0.1.

Perf notes (from neuron-profile traces):
  - Each dma_start trigger occupies the HWDGE ring ~0.7us, so inputs are
    host-packed into a few [128, W] chunk blobs, one DMA each, ordered by
    first use. The first matmul's dependency is a single 0.64MB chunk.
  - A short burst of dummy matmuls runs during the input-DMA head so the
    PE HAM clock gate is already warm (2.4GHz) when real work starts.
  - Stages are interleaved across branches so the matmul stream has no
    eviction-chain stalls (measured: zero PE gaps > 300ns).
"""

import numpy as np
import ml_dtypes

import concourse.bass as bass
import concourse.mybir as mybir
import concourse.tile as tile
from concourse import bacc
from concourse.bass_utils import run_bass_kernel_spmd

B, N = 4, 4096
CH = (64, 128, 256, 512)
NT = N // 2  # tokens per core
P = 128
FD = 512  # matmul moving free dim == one fp32 PSUM bank
N_CORES = 8
N_WARMUP_MM = 9

BF16 = mybir.dt.bfloat16
F32 = mybir.dt.float32
np_bf16 = ml_dtypes.bfloat16

KT = {i: max(C // P, 1) for i, C in enumerate(CH, start=1)}  # k/m tiles
PS = {i: min(C, P) for i, C in enumerate(CH, start=1)}       # partitions/tile

# Packed input chunks: each is one [128, W] bf16 DRAM tensor -> one DMA.
# Pieces are (kind, branch, ktile); widths: wv/wp -> C_i, x -> NT.
CHUNK_SPECS = {
    "c0": [("wv", 4, 0), ("x", 4, 0)],
    "c1": [("wv", 4, 1), ("x", 4, 1)],
    "c2": [("wv", 4, 2), ("x", 4, 2)],
    "c3": [("wv", 4, 3), ("x", 4, 3)],
    "c4": [("wv", 3, 0), ("wv", 3, 1), ("x", 3, 0), ("x", 3, 1)],
    "c5": [("wv", 2, 0), ("x", 2, 0), ("wv", 1, 0), ("x", 1, 0)],
    "cw": [("wp", 4, 0), ("wp", 4, 1), ("wp", 4, 2), ("wp", 4, 3),
           ("wp", 3, 0), ("wp", 3, 1), ("wp", 2, 0), ("wp", 1, 0)],
}
# bias columns in the f32 "bv" chunk: (branch, mtile) -> col
BV_COL = {(4, 0): 0, (4, 1): 1, (4, 2): 2, (4, 3): 3,
          (3, 0): 4, (3, 1): 5, (2, 0): 6, (1, 0): 7}
BV_W = 8


def _piece_width(kind, i):
    return NT if kind == "x" else CH[i - 1]


_OFF = {}  # (kind, branch, k) -> (chunk_name, col_offset)
_CHUNK_W = {}
for _name, _pieces in CHUNK_SPECS.items():
    _off = 0
    for _p in _pieces:
        _OFF[_p] = (_name, _off)
        _off += _piece_width(_p[0], _p[1])
    _CHUNK_W[_name] = _off

_CACHE = {}


def _build_nc():
    """Build the (single, SPMD) Bass program: per-core v-proj + out-proj."""
    nc = bacc.Bacc(None, target_bir_lowering=False, debug=False)

    dram = {}
    for name, w in _CHUNK_W.items():
        dram[name] = nc.dram_tensor(name, [P, w], BF16, kind="ExternalInput")
    dram["bv"] = nc.dram_tensor("bv", [P, BV_W], F32, kind="ExternalInput")
    for i, C in enumerate(CH, start=1):
        dram[f"oT{i}"] = nc.dram_tensor(f"oT{i}", [C, NT], BF16, kind="ExternalOutput")

    with tile.TileContext(nc) as tc:
        with (
            tc.tile_pool(name="inpool", bufs=1) as inpool,
            tc.tile_pool(name="rvpool", bufs=1) as rvpool,
            tc.tile_pool(name="opool", bufs=1) as opool,
            tc.tile_pool(name="psum", bufs=8, space="PSUM") as psum,
        ):
            # PE warm-up: dummy matmuls on a zeroed scratch tile keep the HAM
            # clock gate busy while the first input chunk streams in.
            warm = inpool.tile([P, FD], BF16, name="warm_sb")
            nc.gpsimd.memset(warm[:], 0.0)
            warm_ps = psum.tile([P, FD], F32, name="warm_ps", tag="ps")
            for _ in range(N_WARMUP_MM):
                nc.tensor.matmul(warm_ps[:], warm[:, :P], warm[:], start=True,
                                 stop=True)

            chunks = {}
            for name, w in _CHUNK_W.items():
                chunks[name] = inpool.tile([P, w], BF16, name=f"{name}_sb",
                                           tag=name)
            bv_sb = inpool.tile([P, BV_W], F32, name="bv_sb", tag="bv")

            # One DMA per chunk, issued in consumption order.
            nc.sync.dma_start(chunks["c0"][:], dram["c0"][:])
            nc.sync.dma_start(bv_sb[:], dram["bv"][:])
            for name in ["c1", "c2", "c3", "c4", "c5", "cw"]:
                nc.sync.dma_start(chunks[name][:], dram[name][:])

            def wslice(kind, i, k, m):  # [ps, pm] weight tile for matmul lhsT
                name, off = _OFF[(kind, i, k)]
                pm = min(CH[i - 1] - m * P, P)
                return chunks[name][:PS[i], off + m * P: off + m * P + pm]

            def xslice(i, k, j):  # [ps, FD] moving operand chunk
                name, off = _OFF[("x", i, k)]
                return chunks[name][:PS[i], off + j * FD: off + (j + 1) * FD]

            rv = {}
            osb = {}
            for i, C in enumerate(CH, start=1):
                rv[i] = rvpool.tile([PS[i], KT[i], NT], BF16,
                                    name=f"rv{i}_sb", tag=f"rv{i}")
                osb[i] = opool.tile([PS[i], KT[i], NT], BF16,
                                    name=f"o{i}_sb", tag=f"o{i}")

            # j-outer, k-inner, m-innermost: the first PSUM group only needs
            # the k0 chunk to start, and each arriving k chunk immediately
            # feeds kt matmuls.
            def stage1(i):
                C, kt = CH[i - 1], KT[i]
                for j in range(NT // FD):
                    js = slice(j * FD, (j + 1) * FD)
                    pst = [psum.tile([P, FD], F32, name=f"ps1_{i}_{j}_{m}",
                                     tag="ps") for m in range(kt)]
                    for k in range(kt):
                        for m in range(kt):
                            pm = min(C - m * P, P)
                            nc.tensor.matmul(
                                pst[m][:pm, :], wslice("wv", i, k, m),
                                xslice(i, k, j),
                                start=(k == 0), stop=(k == kt - 1))
                    for m in range(kt):
                        pm = min(C - m * P, P)
                        nc.scalar.activation(
                            rv[i][:pm, m, js], pst[m][:pm, :],
                            mybir.ActivationFunctionType.Relu,
                            bias=bv_sb[:pm, BV_COL[(i, m)]:BV_COL[(i, m)] + 1])

            def stage2(i):
                C, kt = CH[i - 1], KT[i]
                for j in range(NT // FD):
                    js = slice(j * FD, (j + 1) * FD)
                    pst = [psum.tile([P, FD], F32, name=f"ps2_{i}_{j}_{m}",
                                     tag="ps") for m in range(kt)]
                    for k in range(kt):
                        for m in range(kt):
                            pm = min(C - m * P, P)
                            nc.tensor.matmul(
                                pst[m][:pm, :], wslice("wp", i, k, m),
                                rv[i][:, k, js],
                                start=(k == 0), stop=(k == kt - 1))
                    for m in range(kt):
                        pm = min(C - m * P, P)
                        nc.vector.tensor_copy(osb[i][:pm, m, js], pst[m][:pm, :])
                nc.sync.dma_start(
                    dram[f"oT{i}"].rearrange("(k p) n -> p k n", p=PS[i]),
                    osb[i][:])

            # Interleave stages so the PE never waits on an eviction chain:
            # while branch i's stage-2 deps (rv) finish, the next branch's
            # stage-1 matmuls keep the array busy.
            stage1(4)
            stage1(3)
            stage2(4)
            stage1(2)
            stage2(3)
            stage1(1)
            stage2(2)
            stage2(1)

    nc.compile()
    return nc


def _pack_core_inputs(xT, wvT, wp, bv):
    """Pack per-core logical arrays into the chunk blobs the kernel DMAs.

    xT[i]: [C_i, NT] bf16; wvT[i]: [C_i, C_i] bf16; wp[i]: [C_i, C_i] bf16;
    bv[i]: [C_i] f32.
    """
    out = {}
    for name, pieces in CHUNK_SPECS.items():
        blob = np.zeros((P, _CHUNK_W[name]), dtype=np_bf16)
        for kind, i, k in pieces:
            _, off = _OFF[(kind, i, k)]
            w = _piece_width(kind, i)
            src = {"wv": wvT, "wp": wp, "x": xT}[kind][i]
            blob[:PS[i], off:off + w] = src[k * PS[i]:(k + 1) * PS[i], :]
        out[name] = blob
    bvb = np.zeros((P, BV_W), dtype=np.float32)
    for (i, m), col in BV_COL.items():
        pm = min(CH[i - 1] - m * P, P)
        bvb[:pm, col] = bv[i][m * P:m * P + pm]
    out["bv"] = bvb
    return out


def _host_stats(inputs):
    """q -> softmax -> xbar -> ctx, all fp32 on host (~0.3% of FLOPs)."""
    xt = np.asarray(inputs["x_total"], dtype=np.float32)
    q = np.einsum("bnc,c->bn", xt, np.asarray(inputs["q_w"])[0],
                  optimize=True) + np.asarray(inputs["q_b"])[0]
    q -= q.max(axis=1, keepdims=True)
    e = np.exp(q)
    scores = e / e.sum(axis=1, keepdims=True)  # [B, N]
    ctxs = {}
    for i, C in enumerate(CH, start=1):
        x = np.asarray(inputs[f"x{i}"], dtype=np.float32)
        kvw = np.asarray(inputs[f"kv{i}_w"])
        kvb = np.asarray(inputs[f"kv{i}_b"])
        xbar = np.einsum("bnc,bn->bc", x, scores, optimize=True)  # [B, C]
        ctxs[i] = xbar @ kvw[:C].T + kvb[:C]  # [B, C]
    return ctxs


def kernel(**inputs):
    if "nc" not in _CACHE:
        _CACHE["nc"] = _build_nc()
    nc = _CACHE["nc"]

    ctxs = _host_stats(inputs)

    wvT = {}
    bv = {}
    for i, C in enumerate(CH, start=1):
        kvw = np.asarray(inputs[f"kv{i}_w"])
        wvT[i] = np.ascontiguousarray(kvw[C:].T).astype(np_bf16)
        bv[i] = np.asarray(inputs[f"kv{i}_b"])[C:].astype(np.float32)

    in_maps = []
    for core in range(N_CORES):
        b, h = divmod(core, 2)
        xT = {}
        wpd = {}
        for i, C in enumerate(CH, start=1):
            x = np.asarray(inputs[f"x{i}"])
            ow = np.asarray(inputs[f"out{i}_w"])
            xT[i] = np.ascontiguousarray(
                x[b, h * NT:(h + 1) * NT, :].T).astype(np_bf16)
            wpd[i] = (ow.T * ctxs[i][b][:, None]).astype(np_bf16)
        in_maps.append(_pack_core_inputs(xT, wvT, wpd, bv))

    _CACHE["in_maps"] = in_maps
    res = run_bass_kernel_spmd(nc, in_maps, list(range(N_CORES)))
    _CACHE["last_results"] = res

    outs = []
    for i, C in enumerate(CH, start=1):
        ob = np.asarray(inputs[f"out{i}_b"]).astype(np.float32)
        per_b = []
        for b in range(B):
            halves = [res.results[2 * b + h][f"oT{i}"].astype(np.float32)
                      for h in range(2)]
            per_b.append(np.concatenate(halves, axis=1).T + ob)  # [N, C]
        outs.append(np.stack(per_b).astype(np.float32))  # [B, N, C]

    return (outs[0], outs[1], outs[2], outs[3], [])


# revision 5
# speedup vs baseline: 1.2264x; 1.0491x over previous
"""Trainium2 Bass kernel for nn_LinearCrossAttention (B=4, N=4096, CH=(64,128,256,512)).

Math (per sample b, branch i with C channels):
    q      = x_total @ q_w.T + q_b                         [N]
    scores = softmax(q)  over tokens                       [N]
    kv     = x @ kv_w.T + kv_b ; k, v = split(kv)          [N, C] each
    ctx    = sum_n scores[n] * k[n, :]                     [C]
    o      = (relu(v) * ctx) @ out_w.T + out_b             [N, C]

Because scores sums to 1, ctx = kv_w[:C] @ (x.T @ scores) + kv_b[:C] — the
k-projection of every token is never needed, only the pooled xbar = x.T @
scores. And ctx folds into the output projection weights:
    o = relu(v) @ W' + out_b,   W'[c, d] = out_w[d, c] * ctx[c]

Device work = the two big matmuls (v-projection + out-projection, ~22.8
GFLOP total), computed in transposed layout (channels on partitions) so no
on-device transposes are needed. The tiny stats (q, softmax, xbar, ctx,
weight scaling — ~0.3% of FLOPs) run on host while sharding.

Sharding: 8 cores = 4 samples x 2 token-halves. Per core (sample b, half h):
    vT = relu(wvT.T @ xT + bv)      [C, 2048]   (wvT = kv_w[C:2C].T)
    oT = wp.T @ vT                  [C, 2048]   (wp[c,d] = out_w[d,c]*ctx[b,c])
Host gathers oT halves, transposes, adds out_b.

Matmuls run in bf16 (fp32 PSUM accumulation). The out-projection weights are
~0.02-scale and ctx multiplies them further down, so bf16 input rounding
contributes ~1e-5 absolute error against an output scale of ~0.1.

Perf notes (from neuron-profile traces):
  - Each dma_start trigger occupies the HWDGE ring ~0.7us, so inputs are
    host-packed into a few [128, W] chunk blobs, one DMA each, ordered by
    first use. The first matmul's dependency is a single 0.64MB chunk.
  - A short burst of dummy matmuls runs during the input-DMA head so the
    PE HAM clock gate is already warm (2.4GHz) when real work starts.
  - Stages are interleaved across branches so the matmul stream has no
    eviction-chain stalls (measured: zero PE gaps > 300ns).
"""

import numpy as np
import ml_dtypes

import concourse.bass as bass
import concourse.mybir as mybir
import concourse.tile as tile
from concourse import bacc
from concourse.bass_utils import run_bass_kernel_spmd

B, N = 4, 4096
CH = (64, 128, 256, 512)
NT = N // 2  # tokens per core
P = 128
FD = 512  # matmul moving free dim == one fp32 PSUM bank
N_CORES = 8
N_WARMUP_MM = 9

BF16 = mybir.dt.bfloat16
F32 = mybir.dt.float32
np_bf16 = ml_dtypes.bfloat16

KT = {i: max(C // P, 1) for i, C in enumerate(CH, start=1)}  # k/m tiles
PS = {i: min(C, P) for i, C in enumerate(CH, start=1)}       # partitions/tile

# Packed input chunks: each is one [128, W] bf16 DRAM tensor -> one DMA.
# Pieces are (kind, branch, ktile); widths: wv/wp -> C_i, x -> NT.
CHUNK_SPECS = {
    "c0": [("wv", 4, 0), ("x", 4, 0)],
    "c1": [("wv", 4, 1), ("x", 4, 1)],
    "c2": [("wv", 4, 2), ("x", 4, 2)],
    "c3": [("wv", 4, 3), ("x", 4, 3)],
    "c4": [("wv", 3, 0), ("wv", 3, 1), ("x", 3, 0), ("x", 3, 1)],
    "c5": [("wv", 2, 0), ("x", 2, 0), ("wv", 1, 0), ("x", 1, 0)],
    "cw": [("wp", 4, 0), ("wp", 4, 1), ("wp", 4, 2), ("wp", 4, 3),
           ("wp", 3, 0), ("wp", 3, 1), ("wp", 2, 0), ("wp", 1, 0)],
}
# bias columns in the f32 "bv" chunk: (branch, mtile) -> col
BV_COL = {(4, 0): 0, (4, 1): 1, (4, 2): 2, (4, 3): 3,
          (3, 0): 4, (3, 1): 5, (2, 0): 6, (1, 0): 7}
BV_W = 8


def _piece_width(kind, i):
    return NT if kind == "x" else CH[i - 1]


_OFF = {}  # (kind, branch, k) -> (chunk_name, col_offset)
_CHUNK_W = {}
for _name, _pieces in CHUNK_SPECS.items():
    _off = 0
    for _p in _pieces:
        _OFF[_p] = (_name, _off)
        _off += _piece_width(_p[0], _p[1])
    _CHUNK_W[_name] = _off

_CACHE = {}


def _build_nc():
    """Build the (single, SPMD) Bass program: per-core v-proj + out-proj."""
    nc = bacc.Bacc(None, target_bir_lowering=False, debug=False)

    dram = {}
    for name, w in _CHUNK_W.items():
        dram[name] = nc.dram_tensor(name, [P, w], BF16, kind="ExternalInput")
    dram["bv"] = nc.dram_tensor("bv", [P, BV_W], F32, kind="ExternalInput")
    for i, C in enumerate(CH, start=1):
        dram[f"oT{i}"] = nc.dram_tensor(f"oT{i}", [C, NT], BF16, kind="ExternalOutput")

    with tile.TileContext(nc) as tc:
        with (
            tc.tile_pool(name="inpool", bufs=1) as inpool,
            tc.tile_pool(name="rvpool", bufs=1) as rvpool,
            tc.tile_pool(name="opool", bufs=1) as opool,
            tc.tile_pool(name="psum", bufs=8, space="PSUM") as psum,
        ):
            # PE warm-up: dummy matmuls on a zeroed scratch tile keep the HAM
            # clock gate busy while the first input chunk streams in.
            warm = inpool.tile([P, FD], BF16, name="warm_sb")
            nc.gpsimd.memset(warm[:], 0.0)
            warm_ps = psum.tile([P, FD], F32, name="warm_ps", tag="ps")
            for _ in range(N_WARMUP_MM):
                nc.tensor.matmul(warm_ps[:], warm[:, :P], warm[:], start=True,
                                 stop=True)

            chunks = {}
            for name, w in _CHUNK_W.items():
                chunks[name] = inpool.tile([P, w], BF16, name=f"{name}_sb",
                                           tag=name)
            bv_sb = inpool.tile([P, BV_W], F32, name="bv_sb", tag="bv")

            # One DMA per chunk, issued in consumption order.
            nc.sync.dma_start(chunks["c0"][:], dram["c0"][:])
            nc.sync.dma_start(bv_sb[:], dram["bv"][:])
            for name in ["c1", "c2", "c3", "c4", "c5", "cw"]:
                nc.sync.dma_start(chunks[name][:], dram[name][:])

            def wslice(kind, i, k, m):  # [ps, pm] weight tile for matmul lhsT
                name, off = _OFF[(kind, i, k)]
                pm = min(CH[i - 1] - m * P, P)
                return chunks[name][:PS[i], off + m * P: off + m * P + pm]

            def xslice(i, k, j):  # [ps, FD] moving operand chunk
                name, off = _OFF[("x", i, k)]
                return chunks[name][:PS[i], off + j * FD: off + (j + 1) * FD]

            rv = {}
            osb = {}
            for i, C in enumerate(CH, start=1):
                rv[i] = rvpool.tile([PS[i], KT[i], NT], BF16,
                                    name=f"rv{i}_sb", tag=f"rv{i}")
                osb[i] = opool.tile([PS[i], KT[i], NT], BF16,
                                    name=f"o{i}_sb", tag=f"o{i}")

            # j-outer, k-inner, m-innermost: the first PSUM group only needs
            # the k0 chunk to start, and each arriving k chunk immediately
            # feeds kt matmuls.
            def stage1(i):
                C, kt = CH[i - 1], KT[i]
                for j in range(NT // FD):
                    js = slice(j * FD, (j + 1) * FD)
                    pst = [psum.tile([P, FD], F32, name=f"ps1_{i}_{j}_{m}",
                                     tag="ps") for m in range(kt)]
                    for k in range(kt):
                        for m in range(kt):
                            pm = min(C - m * P, P)
                            nc.tensor.matmul(
                                pst[m][:pm, :], wslice("wv", i, k, m),
                                xslice(i, k, j),
                                start=(k == 0), stop=(k == kt - 1))
                    for m in range(kt):
                        pm = min(C - m * P, P)
                        nc.scalar.activation(
                            rv[i][:pm, m, js], pst[m][:pm, :],
                            mybir.ActivationFunctionType.Relu,
                            bias=bv_sb[:pm, BV_COL[(i, m)]:BV_COL[(i, m)] + 1])

            def stage2(i):
                C, kt = CH[i - 1], KT[i]
                oT_d = dram[f"oT{i}"].rearrange("(k p) n -> p k n", p=PS[i])
                for j in range(NT // FD):
                    js = slice(j * FD, (j + 1) * FD)
                    pst = [psum.tile([P, FD], F32, name=f"ps2_{i}_{j}_{m}",
                                     tag="ps") for m in range(kt)]
                    for k in range(kt):
                        for m in range(kt):
                            pm = min(C - m * P, P)
                            nc.tensor.matmul(
                                pst[m][:pm, :], wslice("wp", i, k, m),
                                rv[i][:, k, js],
                                start=(k == 0), stop=(k == kt - 1))
                    for m in range(kt):
                        pm = min(C - m * P, P)
                        nc.vector.tensor_copy(osb[i][:pm, m, js], pst[m][:pm, :])
                    # per-j output DMA: the final branch's last store chain is
                    # one CAST + one small DMA, not the whole branch.
                    nc.sync.dma_start(oT_d[:, :, js], osb[i][:, :, js])

            # Interleave stages so the PE never waits on an eviction chain:
            # while branch i's stage-2 deps (rv) finish, the next branch's
            # stage-1 matmuls keep the array busy.
            stage1(4)
            stage1(3)
            stage2(4)
            stage1(2)
            stage2(3)
            stage1(1)
            stage2(2)
            stage2(1)

    nc.compile()
    return nc


def _pack_core_inputs(xT, wvT, wp, bv):
    """Pack per-core logical arrays into the chunk blobs the kernel DMAs.

    xT[i]: [C_i, NT] bf16; wvT[i]: [C_i, C_i] bf16; wp[i]: [C_i, C_i] bf16;
    bv[i]: [C_i] f32.
    """
    out = {}
    for name, pieces in CHUNK_SPECS.items():
        blob = np.zeros((P, _CHUNK_W[name]), dtype=np_bf16)
        for kind, i, k in pieces:
            _, off = _OFF[(kind, i, k)]
            w = _piece_width(kind, i)
            src = {"wv": wvT, "wp": wp, "x": xT}[kind][i]
            blob[:PS[i], off:off + w] = src[k * PS[i]:(k + 1) * PS[i], :]
        out[name] = blob
    bvb = np.zeros((P, BV_W), dtype=np.float32)
    for (i, m), col in BV_COL.items():
        pm = min(CH[i - 1] - m * P, P)
        bvb[:pm, col] = bv[i][m * P:m * P + pm]
    out["bv"] = bvb
    return out


def _host_stats(inputs):
    """q -> softmax -> xbar -> ctx, all fp32 on host (~0.3% of FLOPs)."""
    xt = np.asarray(inputs["x_total"], dtype=np.float32)
    q = np.einsum("bnc,c->bn", xt, np.asarray(inputs["q_w"])[0],
                  optimize=True) + np.asarray(inputs["q_b"])[0]
    q -= q.max(axis=1, keepdims=True)
    e = np.exp(q)
    scores = e / e.sum(axis=1, keepdims=True)  # [B, N]
    ctxs = {}
    for i, C in enumerate(CH, start=1):
        x = np.asarray(inputs[f"x{i}"], dtype=np.float32)
        kvw = np.asarray(inputs[f"kv{i}_w"])
        kvb = np.asarray(inputs[f"kv{i}_b"])
        xbar = np.einsum("bnc,bn->bc", x, scores, optimize=True)  # [B, C]
        ctxs[i] = xbar @ kvw[:C].T + kvb[:C]  # [B, C]
    return ctxs


def kernel(**inputs):
    if "nc" not in _CACHE:
        _CACHE["nc"] = _build_nc()
    nc = _CACHE["nc"]

    ctxs = _host_stats(inputs)

    wvT = {}
    bv = {}
    for i, C in enumerate(CH, start=1):
        kvw = np.asarray(inputs[f"kv{i}_w"])
        wvT[i] = np.ascontiguousarray(kvw[C:].T).astype(np_bf16)
        bv[i] = np.asarray(inputs[f"kv{i}_b"])[C:].astype(np.float32)

    in_maps = []
    for core in range(N_CORES):
        b, h = divmod(core, 2)
        xT = {}
        wpd = {}
        for i, C in enumerate(CH, start=1):
            x = np.asarray(inputs[f"x{i}"])
            ow = np.asarray(inputs[f"out{i}_w"])
            xT[i] = np.ascontiguousarray(
                x[b, h * NT:(h + 1) * NT, :].T).astype(np_bf16)
            wpd[i] = (ow.T * ctxs[i][b][:, None]).astype(np_bf16)
        in_maps.append(_pack_core_inputs(xT, wvT, wpd, bv))

    _CACHE["in_maps"] = in_maps
    res = run_bass_kernel_spmd(nc, in_maps, list(range(N_CORES)))
    _CACHE["last_results"] = res

    outs = []
    for i, C in enumerate(CH, start=1):
        ob = np.asarray(inputs[f"out{i}_b"]).astype(np.float32)
        per_b = []
        for b in range(B):
            halves = [res.results[2 * b + h][f"oT{i}"].astype(np.float32)
                      for h in range(2)]
            per_b.append(np.concatenate(halves, axis=1).T + ob)  # [N, C]
        outs.append(np.stack(per_b).astype(np.float32))  # [B, N, C]

    return (outs[0], outs[1], outs[2], outs[3], [])
